# revision 4
# baseline (speedup 1.0000x reference)
"""Trainium2 Bass kernel for MHSA with relative position bias (nn_MHSARPB).

Problem (hardcoded): x (8, 32, 32, 512), qkv_w (1536, 512), qkv_b (1536,),
rpb (16, 63, 63), proj_w (512, 512), proj_b (512,). Output (8, 32, 32, 512) f32.

Strategy: tensor-parallel over the 16 heads -> 2 heads per core on 8 cores.
Each core computes q/k/v for its 2 heads, full attention for its 16 (b, h)
pairs, and a partial projection output (contraction over its 64 channels).
Host sums the 8 partial outputs and adds proj_b.

v2 structural changes vs the 397us baseline:
  - Softmax-exp pipelined on 2-bank [128,1024] PSUM ping-pong tiles so the
    scalar engine (the 121us roofline for 16.8M exps/core) runs back-to-back.
  - Per-batch denominator path: ones-columns placed so the two heads'
    denominator rows land adjacent (63/64), one DVE reciprocal, two gpsimd
    partition_broadcasts, and one fused (PSUM-drain x recip) DVE op.
    No DRAM round-trips, no global barrier.
  - Projection interleaved per batch into the next batch's attention
    (PSUM slot shared with the v matmul via one flex tag).
  - k-bias dropped exactly (it only shifts logits by a per-query constant
    which softmax cancels); q keeps scale+bias fused into its drain.
  - v transpose-scatter as one SWDGE descriptor per batch instead of 8.
"""
import sys

sys.path.insert(0, "/opt/trn_rl_repo")

import contextlib
import numpy as np
import concourse.bass as bass
import concourse.bacc as bacc
import concourse.tile as tile
from concourse import mybir
from concourse.bass_utils import run_bass_kernel_spmd

FP16 = mybir.dt.float16
FP32 = mybir.dt.float32
EXP = mybir.ActivationFunctionType.Exp
MULT = mybir.AluOpType.mult

B, S, C, NH = 8, 32, 512, 16
N = S * S            # 1024 tokens per image
T = B * N            # 8192 tokens
D = C // NH          # 32 head dim
SCALE = D ** -0.5
N_CORES = 8

_CACHE = {}


def build_nc(repeat=1):
    nc = bacc.Bacc("TRN2", target_bir_lowering=False, debug=False)

    xT = nc.dram_tensor("xT", [C, T], FP16, kind="ExternalInput")
    wqkT = nc.dram_tensor("wqkT", [4, 128, 128], FP16, kind="ExternalInput")
    wvT = nc.dram_tensor("wvT", [4, 128, 64], FP16, kind="ExternalInput")
    bqk = nc.dram_tensor("bqk", [128, 1], FP32, kind="ExternalInput")
    bv2 = nc.dram_tensor("bv2", [128, 1], FP32, kind="ExternalInput")
    expb = nc.dram_tensor("expb", [128, 16384], FP16, kind="ExternalInput")
    projT = nc.dram_tensor("projT", [128, 512], FP16, kind="ExternalInput")
    outT = nc.dram_tensor("outT", [C, T], FP16, kind="ExternalOutput")
    den_scr = nc.dram_tensor("den_scr", [16, 1024], FP16)

    with tile.TileContext(nc) as tc:
        with (
            tc.For_i(0, repeat, 1) if repeat > 1 else contextlib.nullcontext(),
            tc.tile_pool(name="consts", bufs=1) as consts,
            tc.tile_pool(name="big", bufs=1) as big,
            tc.tile_pool(name="xin", bufs=3) as xin,
            tc.tile_pool(name="qkp", bufs=2) as qkp,
            tc.tile_pool(name="epool", bufs=6) as epool,
            tc.tile_pool(name="vstg", bufs=2) as vstg,
            tc.tile_pool(name="ypool", bufs=3) as ypool,
            tc.tile_pool(name="rpool", bufs=2) as rpool,
            tc.tile_pool(name="opool", bufs=3) as opool,
        ):
            # ---- constants -------------------------------------------------
            wqk_sb = consts.tile([128, 4 * 128], FP16, tag="wqk_sb")
            nc.sync.dma_start(
                out=wqk_sb.rearrange("p (kc f) -> p kc f", kc=4),
                in_=wqkT[:].transpose([1, 0, 2]),
            )
            wv_sb = consts.tile([128, 4 * 64], FP16, tag="wv_sb")
            nc.sync.dma_start(
                out=wv_sb.rearrange("p (kc f) -> p kc f", kc=4),
                in_=wvT[:].transpose([1, 0, 2]),
            )
            bqk_sb = consts.tile([128, 1], FP32, tag="bqk_sb")
            nc.sync.dma_start(out=bqk_sb[:], in_=bqk[:])
            bv2_sb = consts.tile([128, 1], FP32, tag="bv2_sb")
            nc.sync.dma_start(out=bv2_sb[:], in_=bv2[:])
            expb_sb = consts.tile([128, 16384], FP16, tag="expb_sb")
            nc.sync.dma_start(out=expb_sb[:], in_=expb[:])
            projT_sb = consts.tile([128, 512], FP16, tag="projT_sb")
            nc.sync.dma_start(out=projT_sb[:], in_=projT[:])

            # v in natural token-major layout: per (hi, b, j) a 64-col block:
            # d at cols 0:32, ones at col 32 -> av d rows 64hi+[0,32), den row
            # 64hi+32 (aligned: engine ops cannot start at partition 63)
            v_nat = big.tile([128, T], FP16, tag="v_nat")
            nc.gpsimd.memset(v_nat[:], 0.0)
            v5 = v_nat.rearrange("p (hi b j col) -> p hi b j col", hi=2, b=8, j=8)
            nc.gpsimd.memset(v5[:, :, :, :, 32:33], 1.0)

            with (
                tc.tile_pool(name="ps_s", bufs=2, space="PSUM") as ps_s,
                tc.tile_pool(name="ps_qk", bufs=1, space="PSUM") as ps_qk,
                tc.tile_pool(name="ps_fx", bufs=1, space="PSUM") as ps_fx,
                tc.tile_pool(name="ps_av", bufs=1, space="PSUM") as ps_av,
            ):
                state = {}

                def emit_qkv(b):
                    """qkv matmuls + drains for batch b (xt must be loaded)."""
                    xt = state.pop(("xt", b))
                    qkT = qkp.tile([128, 1024], FP16, tag="qkT", name=f"qkT_{b}")
                    for cc in range(2):
                        psqk = ps_qk.tile([128, 512], FP32, tag="psqk",
                                          name=f"psqk_{b}_{cc}")
                        for kc in range(4):
                            nc.tensor.matmul(
                                psqk[:],
                                wqk_sb[:, kc * 128: (kc + 1) * 128],
                                xt[:, kc * 1024 + cc * 512: kc * 1024 + cc * 512 + 512],
                                start=(kc == 0), stop=(kc == 3),
                            )
                        nc.vector.tensor_scalar_add(
                            qkT[:, cc * 512: (cc + 1) * 512], psqk[:], bqk_sb[:]
                        )
                    vps = ps_fx.tile([128, 512], FP32, tag="flex", name=f"psv_{b}")
                    for cc in range(2):
                        for kc in range(4):
                            nc.tensor.matmul(
                                vps[64 * cc: 64 * cc + 64, :],
                                wv_sb[:, kc * 64: (kc + 1) * 64],
                                xt[:, kc * 1024 + cc * 512: kc * 1024 + cc * 512 + 512],
                                start=(kc == 0), stop=(kc == 3),
                                tile_position=(0, 64 * cc),
                            )
                    vT = vstg.tile([128, 512], FP16, tag="vT", name=f"vT_{b}")
                    nc.vector.tensor_scalar_add(vT[:], vps[:], bv2_sb[:])

                    # v transpose (4 HWDGE xbar transposes) + 2 SWDGE scatters
                    stg = vstg.tile([128, 512], FP16, tag="stg", name=f"stg_{b}")
                    for wl in range(4):
                        nc.sync.dma_start_transpose(
                            out=stg[:, wl * 128: (wl + 1) * 128],
                            in_=vT[:, wl * 128: (wl + 1) * 128],
                        )
                    # v5 block j = wl + 4*par holds tokens [j*128, j*128+128);
                    # stg free layout is (wl, par, hi, d) so iterate (par, wl)
                    # to match ascending j. hi0 d -> cols 0:32, hi1 -> 1:33.
                    stg5 = stg.rearrange("p (wl par hi d) -> p wl par hi d",
                                         wl=4, par=2, hi=2)
                    v6 = v_nat.rearrange(
                        "p (hi b par wl col) -> p hi b wl par col",
                        hi=2, b=8, par=2, wl=4,
                    )
                    for hi in range(2):
                        for par in range(2):
                            nc.gpsimd.dma_start(
                                out=v6[:, hi, b, :, par, 0:32],
                                in_=stg5[:, :, par, hi, :],
                            )

                    # kT_pack: keys to row bands (u = 256-token chunk);
                    # qrep: q replicated to all 4 bands.
                    kT_pack = qkp.tile([128, 512], FP16, tag="kT_pack",
                                       name=f"kTp_{b}")
                    for hi in range(2):
                        for u in range(4):
                            nc.sync.dma_start(
                                out=kT_pack[32 * u: 32 * u + 32,
                                            hi * 256: hi * 256 + 256],
                                in_=qkT[64 + 32 * hi: 64 + 32 * hi + 32,
                                        u * 256: u * 256 + 256],
                            )
                    qreps = []
                    for hi in range(2):
                        q_t = qkp.tile([128, 1024], FP16, tag=f"qrep{hi}",
                                       name=f"qrep_{b}_{hi}")
                        for u in range(4):
                            nc.scalar.dma_start(
                                out=q_t[32 * u: 32 * u + 32, :],
                                in_=qkT[32 * hi: 32 * hi + 32, :],
                            )
                        qreps.append(q_t)
                    state[("qrep", b)] = qreps
                    state[("kT_pack", b)] = kT_pack

                def emit_proj_tile(b, i):
                    """One projection tile (cs, tchunk) for batch b."""
                    y = state[("y", b)]
                    cs, t = i // 2, i % 2
                    pj = ps_fx.tile([128, 512], FP32, tag="flex",
                                    name=f"pj_{b}_{i}")
                    nc.tensor.matmul(
                        pj[:],
                        projT_sb[:, cs * 128: (cs + 1) * 128],
                        y[:, t * 512: (t + 1) * 512],
                        start=True, stop=True,
                    )
                    o_t = opool.tile([128, 512], FP16, tag="o_t",
                                     name=f"ot_{b}_{i}")
                    if i % 4 == 3:
                        nc.scalar.copy(o_t[:], pj[:])
                    else:
                        nc.vector.tensor_copy(o_t[:], pj[:])
                    nc.scalar.dma_start(
                        out=outT[cs * 128: (cs + 1) * 128,
                                 b * 1024 + t * 512: b * 1024 + t * 512 + 512],
                        in_=o_t[:],
                    )

                def emit_attention(b):
                    qreps = state.pop(("qrep", b))
                    kT_pack = state.pop(("kT_pack", b))
                    av = ps_av.tile([128, 1024], FP32, tag="av", name=f"av_{b}")
                    region_cnt = {}
                    av_pending = []

                    def emit_av(e, half, hi, jj, uu):
                        ev = e.rearrange("p (s n) -> p s n", n=512)
                        for s in range(2):
                            kb = 2 * (2 * uu + s) + jj
                            cnt = region_cnt.get((half, hi), 0)
                            region_cnt[(half, hi)] = cnt + 1
                            nc.tensor.matmul(
                                av[64 * hi: 64 * hi + 64,
                                   half * 512: half * 512 + 512],
                                v5[:, hi, b, kb, :],
                                ev[:, s, :],
                                start=(cnt == 0), stop=(cnt == 7),
                                tile_position=(0, 64 * hi),
                                skip_group_check=True,
                            )

                    groups = [(half, hi, jj, uu)
                              for half in range(2) for hi in range(2)
                              for jj in range(2) for uu in range(2)]
                    for idx, (half, hi, jj, uu) in enumerate(groups):
                        sps = ps_s.tile([128, 1024], FP32, tag="sps",
                                        name=f"sps_{b}_{idx}")
                        spsv = sps.rearrange("p (s n) -> p s n", n=512)
                        for s in range(2):
                            u = 2 * uu + s
                            nc.tensor.matmul(
                                spsv[:, s, :],
                                kT_pack[32 * u: 32 * u + 32,
                                        hi * 256 + jj * 128:
                                        hi * 256 + jj * 128 + 128],
                                qreps[hi][32 * u: 32 * u + 32,
                                          half * 512: half * 512 + 512],
                                start=True, stop=True,
                                tile_position=(32 * u, 0),
                            )
                        e = epool.tile([128, 1024], FP16, tag="E",
                                       name=f"e_{b}_{idx}")
                        nc.scalar.activation(e[:], sps[:], EXP)
                        blk = ((hi * 2 + half) * 2 + jj) * 2 + uu
                        nc.vector.tensor_mul(
                            e[:], e[:],
                            expb_sb[:, blk * 1024: (blk + 1) * 1024],
                        )
                        # AV matmuls lag the QK stream by 2 groups so the PE
                        # never waits in-order on an exp/mul that hasn't run
                        av_pending.append((e, half, hi, jj, uu))
                        if idx % 2 == 1 and len(av_pending) > 2:
                            emit_av(*av_pending.pop(0))
                            emit_av(*av_pending.pop(0))
                        # interleave: previous batch's projection + next
                        # batch's qkv, spread through this batch's attention
                        if idx % 2 == 1 and b > 0:
                            emit_proj_tile(b - 1, idx // 2)
                        if idx == 7 and b < 7:
                            emit_qkv(b + 1)
                    while av_pending:
                        emit_av(*av_pending.pop(0))

                    # normalize: recip of adjacent den rows 63/64, broadcast
                    # per 64-row half, fused drain (av * recip) -> y fp16
                    # copy raw den rows out (fp16), bounce through DRAM
                    # with a stride-0 partition AP, then one all-SBUF
                    # reciprocal over the broadcast tile (2x DVE mode)
                    r2 = rpool.tile([128, 1024], FP16, tag="r2", name=f"r2_{b}")
                    nc.vector.tensor_copy(r2[32:33, :], av[32:33, :])
                    nc.vector.tensor_copy(r2[96:97, :], av[96:97, :])
                    nc.scalar.dma_start(
                        out=den_scr[2 * b: 2 * b + 1, :], in_=r2[32:33, :])
                    nc.scalar.dma_start(
                        out=den_scr[2 * b + 1: 2 * b + 2, :], in_=r2[96:97, :])
                    rb = rpool.tile([128, 1024], FP16, tag="rb", name=f"rb_{b}")
                    for s in range(2):
                        nc.sync.dma_start(
                            out=rb[64 * s: 64 * s + 64, :],
                            in_=bass.AP(
                                tensor=den_scr,
                                offset=(2 * b + s) * 1024,
                                ap=[[0, 64], [1, 1024]],
                            ),
                        )
                    with nc.allow_low_precision(
                        reason="softmax denominators are O(1000); fp16 recip "
                               "adds ~5e-4 rel err, well inside tolerance"
                    ):
                        nc.vector.reciprocal(rb[:], rb[:])
                    y = ypool.tile([128, 1024], FP16, tag="y", name=f"y_{b}")
                    nc.vector.scalar_tensor_tensor(
                        out=y[:], in0=av[:], scalar=1.0, in1=rb[:],
                        op0=MULT, op1=MULT,
                    )
                    state[("y", b)] = y

                def load_x(b):
                    xt = xin.tile([128, 4096], FP16, tag="xt", name=f"xt_{b}")
                    nc.sync.dma_start(
                        out=xt.rearrange("p (kc f) -> p kc f", kc=4),
                        in_=xT.rearrange("(kc p) t -> p kc t", p=128)[
                            :, :, b * 1024: (b + 1) * 1024
                        ],
                    )
                    state[("xt", b)] = xt

                # ---- main pipeline ---------------------------------------
                load_x(0)
                load_x(1)
                emit_qkv(0)
                for b in range(8):
                    if b + 2 < 8:
                        load_x(b + 2)
                    emit_attention(b)  # also emits proj(b-1), qkv(b+1)
                for i in range(8):
                    emit_proj_tile(7, i)

    nc.compile()
    return nc


def _relative_position_index(Ssz):
    coords = np.stack(np.meshgrid(np.arange(Ssz), np.arange(Ssz), indexing='ij'))
    coords = coords.reshape(2, -1)
    rel = coords[:, :, None] - coords[:, None, :]
    rel = rel.transpose(1, 2, 0).astype(np.int64)
    rel[:, :, 0] += Ssz - 1
    rel[:, :, 1] += Ssz - 1
    rel[:, :, 0] *= 2 * Ssz - 1
    idx = rel.sum(-1)
    return idx[::-1, ::-1].copy()


def _prep_inputs(x, qkv_w, qkv_b, rpb, proj_w, proj_b):
    x = np.asarray(x, np.float32)
    qkv_w = np.asarray(qkv_w, np.float32)
    qkv_b = np.asarray(qkv_b, np.float32)
    rpb = np.asarray(rpb, np.float32)
    proj_w = np.asarray(proj_w, np.float32)

    xT16 = np.ascontiguousarray(x.reshape(T, C).T).astype(np.float16)
    mi = (np.arange(N) // S)[:, None]
    mj = (np.arange(N) % S)[:, None]
    ni = (np.arange(N) // S)[None, :]
    nj = (np.arange(N) % S)[None, :]

    in_maps = []
    for core in range(N_CORES):
        h0, h1 = 2 * core, 2 * core + 1
        rq = list(range(h0 * D, h0 * D + D)) + list(range(h1 * D, h1 * D + D))
        wq = qkv_w[rq, :] * SCALE
        wk = qkv_w[[C + r for r in rq], :]
        wv = qkv_w[[2 * C + r for r in rq], :]
        bq = qkv_b[rq] * SCALE
        bv = qkv_b[[2 * C + r for r in rq]]

        wqk = np.concatenate([wq, wk], axis=0)           # (128, 512)
        wqkT16 = np.ascontiguousarray(wqk.T).astype(np.float16).reshape(4, 128, 128)
        wvT16 = np.ascontiguousarray(wv.T).astype(np.float16).reshape(4, 128, 64)
        # k-bias dropped (exact: softmax cancels its per-query logit shift)
        bqk_in = np.concatenate([bq, np.zeros(64, np.float32)]).reshape(128, 1)
        bv2_in = np.concatenate([bv, bv]).astype(np.float32).reshape(128, 1)

        # expb tile (hi, half, jj, uu): [128, (s2 x 512q)] with key token
        # m = (2uu+s)*256 + jj*128 + p, query n = half*512 + q
        expb_in = np.zeros((128, 16384), np.float16)
        for hi, h in enumerate((h0, h1)):
            biasT = np.exp(rpb[h][31 + mi - ni, 31 + mj - nj]).astype(np.float16)
            for half in range(2):
                for jj in range(2):
                    for uu in range(2):
                        blk = ((hi * 2 + half) * 2 + jj) * 2 + uu
                        for s in range(2):
                            m0 = (2 * uu + s) * 256 + jj * 128
                            expb_in[:, blk * 1024 + s * 512: blk * 1024 + s * 512 + 512] = \
                                biasT[m0: m0 + 128, half * 512: half * 512 + 512]

        projT_in = np.zeros((128, 512), np.float16)
        projT_in[0:32] = proj_w[:, 64 * core: 64 * core + 32].T.astype(np.float16)
        projT_in[64:96] = proj_w[:, 64 * core + 32: 64 * core + 64].T.astype(np.float16)

        in_maps.append({
            "xT": xT16,
            "wqkT": wqkT16,
            "wvT": wvT16,
            "bqk": bqk_in,
            "bv2": bv2_in,
            "expb": expb_in,
            "projT": projT_in,
        })
    return in_maps


def kernel(x, qkv_w, qkv_b, rpb, proj_w, proj_b):
    if "nc" not in _CACHE:
        _CACHE["nc"] = build_nc()
    nc = _CACHE["nc"]
    in_maps = _prep_inputs(x, qkv_w, qkv_b, rpb, proj_w, proj_b)
    res = run_bass_kernel_spmd(nc, in_maps, list(range(N_CORES)))
    out = np.zeros((T, C), np.float32)
    for core in range(N_CORES):
        out += res.results[core]["outT"].astype(np.float32).T
    out += np.asarray(proj_b, np.float32)[None, :]
    return out.reshape(B, S, S, C)


if __name__ == "__main__":
    rng = np.random.default_rng(0)
    ins = {
        "x": rng.standard_normal((B, S, S, C)).astype(np.float32),
        "qkv_w": (rng.standard_normal((3 * C, C)) * 0.02).astype(np.float32),
        "qkv_b": (rng.standard_normal((3 * C,)) * 0.02).astype(np.float32),
        "rpb": (rng.standard_normal((NH, 2 * S - 1, 2 * S - 1)) * 0.02).astype(np.float32),
        "proj_w": (rng.standard_normal((C, C)) * 0.02).astype(np.float32),
        "proj_b": (rng.standard_normal((C,)) * 0.02).astype(np.float32),
    }
    out = kernel(**ins)
    print("kernel ran, out", out.shape, out.dtype, float(np.abs(out).max()))
